# revision 5
# baseline (speedup 1.0000x reference)
"""BoundaryLoss: single big shape-register indirect gather.

Per core (8192 rows): host precomputes flat element offsets
(row*G + label), laid out channel-wrapped (element i at [i%128, i//128]).
One indirect_dma_start with a 3-D out AP [1, 8192, 1] triggers the
shape-register indirect1d encoding: 8192 indices, 4 B each, landing in
partition 0. A reshape DMA spreads them to [128, 64]; a fused
tensor_scalar (sub margin, min 0, accum_out) yields [128,1] partials;
host sums and negates. Sum is invariant to the gather's landing order.
"""

import os
import sys

for _p in ("/opt/trn_rl_repo", os.path.expanduser("~/.axon_site/_ro/trn_rl_repo")):
    if os.path.isdir(_p) and _p not in sys.path:
        sys.path.insert(0, _p)

import numpy as np

import concourse.bacc as bacc
import concourse.bass as bass
import concourse.mybir as mybir
import concourse.tile as tile
from concourse import bass_utils

POSITIVE_MARGIN = 0.99999
N, G = 65536, 1024
NCORES = 8
NS = N // NCORES
P = 128
F = NS // P  # 64

N_IND = 1          # number of indirect instructions (split of the 8192 idxs)
SBUF_RESHAPE = True  # SBUF->SBUF reshape DMA; False = DRAM bounce


def build_program(n_ind: int = N_IND, sbuf_reshape: bool = SBUF_RESHAPE):
    f32 = mybir.dt.float32
    i32 = mybir.dt.int32
    per = NS // n_ind          # idxs per instruction
    wper = F // n_ind          # offs words per channel per instruction

    nc = bacc.Bacc(
        "TRN2",
        target_bir_lowering=False,
        debug=False,
        dynamic_dma_scratch_size=32768,
    )
    x_t = nc.dram_tensor("inputs", [NS, G], f32, kind="ExternalInput")
    off_t = nc.dram_tensor("offsets", [P, F], i32, kind="ExternalInput")
    out_t = nc.dram_tensor("partials", [P, 1], f32, kind="ExternalOutput")
    if not sbuf_reshape:
        g_t = nc.dram_tensor("gbounce", [NS], f32, kind="Internal")

    with tile.TileContext(nc) as tc:
        with tc.tile_pool(name="pool", bufs=1) as pool:
            offs = pool.tile([P, F], i32)
            nc.sync.dma_start(out=offs[:], in_=off_t.ap())

            v1 = pool.tile([n_ind, per], f32)
            for j in range(n_ind):
                nc.gpsimd.indirect_dma_start(
                    out=v1[j : j + 1, :].rearrange("q (a b) -> q a b", b=1),
                    out_offset=None,
                    in_=x_t.ap(),
                    in_offset=bass.IndirectOffsetOnAxis(
                        ap=offs[:, j * wper : (j + 1) * wper], axis=1
                    ),
                )

            re = pool.tile([P, F], f32)
            if sbuf_reshape:
                nc.sync.dma_start(out=re[:], in_=v1[:])
            else:
                nc.sync.dma_start(out=g_t.ap(), in_=v1[:])
                nc.sync.dma_start(
                    out=re[:], in_=g_t.ap().rearrange("(p w) -> p w", p=P)
                )

            clamp_t = pool.tile([P, F], f32)
            acc = pool.tile([P, 1], f32)
            nc.vector.tensor_scalar(
                out=clamp_t[:],
                in0=re[:],
                scalar1=POSITIVE_MARGIN,
                scalar2=0.0,
                op0=mybir.AluOpType.subtract,
                op1=mybir.AluOpType.min,
            )
            nc.vector.reduce_sum(acc[:], clamp_t[:], axis=mybir.AxisListType.X)
            nc.scalar.dma_start(out=out_t.ap(), in_=acc[:])

    nc.compile()
    return nc


_PROG = None


def _get_prog():
    global _PROG
    if _PROG is None:
        _PROG = build_program()
    return _PROG


_ROWBASE = (np.arange(NS, dtype=np.int64) * G).astype(np.int32)


def _make_in_maps(inputs: np.ndarray, labels: np.ndarray):
    inputs = np.asarray(inputs)
    labels = np.asarray(labels)
    assert inputs.shape == (N, G), inputs.shape
    assert labels.shape == (N,), labels.shape
    inputs = np.ascontiguousarray(inputs, dtype=np.float32)
    lab32 = labels.astype(np.int32)

    in_maps = []
    for c in range(NCORES):
        sl = slice(c * NS, (c + 1) * NS)
        flat = _ROWBASE + lab32[sl]
        # channel-wrapped per instruction block: within block j, element i
        # sits at [i % 128, j*wper + i // 128]
        wper = F // N_IND
        per = NS // N_IND
        blocks = [
            flat[j * per : (j + 1) * per].reshape(wper, P).T
            for j in range(N_IND)
        ]
        offs = np.concatenate(blocks, axis=1).astype(np.int32)
        in_maps.append(
            {"inputs": inputs[sl], "offsets": np.ascontiguousarray(offs)}
        )
    return in_maps


def _run(inputs, labels, trace: bool = False):
    nc = _get_prog()
    in_maps = _make_in_maps(inputs, labels)
    res = bass_utils.run_bass_kernel_spmd(
        nc, in_maps, core_ids=list(range(NCORES)), trace=trace
    )
    total = 0.0
    for r in res.results:
        total += float(np.asarray(r["partials"], dtype=np.float64).sum())
    out = np.array(-total / N, dtype=np.float32)
    return out, res


def kernel(inputs, labels):
    out, _ = _run(inputs, labels, trace=False)
    return out
